# revision 18
# baseline (speedup 1.0000x reference)
"""Causal self-attention (B=2, T=2048, C=1024, H=16, D=64) on 8 TRN2 NeuronCores.

Tensor-parallel over heads: each core owns 2 heads (w_qkv columns / w_out rows
sharded by head, x replicated). Each core computes qkv -> causal attention ->
partial output projection; the host sums the 8 fp16 partials and adds b_out.

Perf design:
  * Scores for the two local heads are computed CONCURRENTLY on the PE array
    via row tiling (K=64 each, tile_position (0,0)/(64,0)) -> 2x score rate.
  * One Exp activation per key-tile round covers both heads ([128,2,512]
    strided AP); ACT does nothing but exp.
  * V^T is produced directly by matmul (x-tile as stationary operand) -- no
    PE transposes; the PSUM->SBUF copy doubles as the v-bias add (DVE).
  * Softmax denominators ride as a ones-column in the PV matmul; both heads'
    PV accumulators live in one [65,2,512] PSUM pair so the reciprocal is a
    single DVE op straight off PSUM row 64, broadcast from partition 64.
  * Output partials are written fp16 (halves HBM traffic; host sums in fp32).
  * Emission is software-pipelined per 512-token block tb: the attention
    rounds of tb are interleaved with out-proj(tb-1) and qkv(tb+1) matmuls
    so the PE instruction stream never stalls on exp and the HAM clock gate
    stays at full rate.
  * PSUM budget (8 banks): scores 2x[128,2x512] (4) + pv [65,2x512] (2)
    + shared projection/out-proj pool 2x[128,512] (2).
"""

import numpy as np

import concourse.bass as bass
from concourse import bacc
import concourse.bass_utils as bass_utils
import concourse.mybir as mybir
from concourse.tile import TileContext

B, T, C, H, D = 2, 2048, 1024, 16, 64
BT = B * T
NCORES = 8
HPC = H // NCORES          # heads per core = 2
JL = 3 * HPC * D           # 384 local qkv output columns
CL = HPC * D               # 128 local channels into out-proj
KT = 128                   # keys per tile (partition dim of scores^T)
QB = 512                   # queries per block (free dim of scores^T)
NQB = T // QB              # 4 query blocks per batch
NTB = BT // QB             # 8 token blocks total
F32 = mybir.dt.float32
F16 = mybir.dt.float16
AF = mybir.ActivationFunctionType

_cache = {}


def _build_bass():
    nc = bacc.Bacc("TRN2", target_bir_lowering=False, debug=False)
    xT = nc.dram_tensor("xT", [C, BT], F16, kind="ExternalInput").ap()
    wqkv = nc.dram_tensor("wqkv", [C, JL], F16, kind="ExternalInput").ap()
    bqk = nc.dram_tensor("bqk", [128, 2], F32, kind="ExternalInput").ap()
    bv16 = nc.dram_tensor("bv16", [1, 128], F16, kind="ExternalInput").ap()
    wout = nc.dram_tensor("wout", [CL, C], F16, kind="ExternalInput").ap()
    outp = nc.dram_tensor("outp", [BT, C], F16, kind="ExternalOutput").ap()

    with TileContext(nc) as tc:
        with (
            tc.tile_pool(name="const", bufs=1) as const,
            tc.tile_pool(name="xtp", bufs=3) as xtp,
            tc.tile_pool(name="ptp", bufs=4) as ptp,
            tc.tile_pool(name="stg", bufs=2) as stg,
            tc.tile_pool(name="rbp", bufs=2) as rbp,
            tc.tile_pool(name="obp", bufs=4) as obp,
            tc.tile_pool(name="psS", bufs=2, space="PSUM") as psS,
            tc.tile_pool(name="psPV", bufs=1, space="PSUM") as psPV,
            tc.tile_pool(name="psA", bufs=2, space="PSUM") as psA,
        ):
            # ---- static tensors (w_qk + first x tiles issued first: they
            # gate the first matmuls)
            w_sb = const.tile([128, 8, JL], F16)
            wr = wqkv.rearrange("(k p) j -> p k j", p=128)
            nc.sync.dma_start(out=w_sb[:, :, 0:256], in_=wr[:, :, 0:256])

            xt_t = {}

            def emit_xt_dma(tb):
                xr = xT[:, tb * QB:(tb + 1) * QB].rearrange(
                    "(k p) t -> p k t", p=128)
                xa = xtp.tile([128, 4, QB], F16, tag="xta", name="xta")
                nc.sync.dma_start(out=xa, in_=xr[:, 0:4, :])
                xb = xtp.tile([128, 4, QB], F16, tag="xtb", name="xtb")
                nc.gpsimd.dma_start(out=xb, in_=xr[:, 4:8, :])
                xt_t[tb] = (xa, xb)

            def xchunk(tb, k):
                return xt_t[tb][k // 4][:, k % 4, :]

            emit_xt_dma(0)
            bqk_sb = const.tile([128, 2], F32)
            nc.sync.dma_start(out=bqk_sb, in_=bqk)
            nc.sync.dma_start(out=w_sb[:, :, 256:384], in_=wr[:, :, 256:384])
            emit_xt_dma(1)
            wout_sb = const.tile([128, C], F16)
            nc.sync.dma_start(out=wout_sb, in_=wout)
            bvrow = const.tile([1, 128], F16)
            nc.sync.dma_start(out=bvrow, in_=bv16)
            vb_bcast = const.tile([128, 128], F16)
            nc.gpsimd.partition_broadcast(vb_bcast, bvrow)
            ones_row = const.tile([1, 128], F16)
            nc.vector.memset(ones_row, 1.0)
            qT = const.tile([128, BT], F16)    # rows: [h0 d64 | h1 d64]
            kTt = const.tile([128, BT], F16)
            # V in [t, d] tiles + ones column for softmax denominators
            v_sb = const.tile([128, HPC, B, T // KT, D + 1], F16)
            for h in range(HPC):
                for b_ in range(B):
                    nc.vector.memset(v_sb[:, h, b_, :, D:D + 1], 1.0)
            attnTc = const.tile([128, BT], F16)
            attnT1 = const.tile([64, BT], F16)

            def emit_qk(tb, m, on_act=False):
                ps = psA.tile([128, QB], F32, tag="pa", name="psqk")
                for k in range(8):
                    nc.tensor.matmul(
                        ps,
                        lhsT=w_sb[:, k, m * 128:(m + 1) * 128],
                        rhs=xchunk(tb, k),
                        start=(k == 0), stop=(k == 7))
                dst = (qT if m == 0 else kTt)[:, tb * QB:(tb + 1) * QB]
                if on_act:
                    nc.scalar.activation(out=dst, in_=ps, func=AF.Identity,
                                         bias=bqk_sb[:, m:m + 1])
                else:
                    nc.vector.tensor_scalar_add(dst, ps, bqk_sb[:, m:m + 1])

            def emit_vT(tb, c4):
                b_ = tb // NQB
                kt = (tb % NQB) * (QB // KT) + c4
                ps = psA.tile([128, 128], F32, tag="pa", name="psv")
                for k in range(8):
                    nc.tensor.matmul(
                        ps,
                        lhsT=xchunk(tb, k)[:, c4 * 128:(c4 + 1) * 128],
                        rhs=w_sb[:, k, 256:384],
                        start=(k == 0), stop=(k == 7))
                for h in range(HPC):
                    nc.vector.tensor_add(
                        v_sb[:, h, b_, kt, 0:D],
                        ps[:, h * 64:(h + 1) * 64],
                        vb_bcast[:, h * 64:(h + 1) * 64])

            def emit_round(b_, qb, kt, pvt, n_kt):
                """One key-tile round: packed scores -> exp -> (mask) -> PV."""
                q0 = b_ * T + qb * QB
                diag = kt >= qb * (QB // KT)
                off = KT * (kt - qb * (QB // KT)) if diag else 0
                w = QB - off
                ks = slice(b_ * T + kt * KT, b_ * T + (kt + 1) * KT)
                ps = psS.tile([128, 2, QB], F32, tag="ps", name="ps")
                for h in range(HPC):
                    hs = slice(h * 64, (h + 1) * 64)
                    nc.tensor.matmul(
                        ps[:, h, 0:w],
                        lhsT=kTt[hs, ks],
                        rhs=qT[hs, q0 + off:q0 + QB],
                        start=True, stop=True,
                        tile_position=(h * 64, 0))
                pt = ptp.tile([128, 2, QB], F16, tag="pt", name="pt")
                nc.scalar.activation(
                    out=pt[:, :, 0:w], in_=ps[:, :, 0:w],
                    func=AF.Exp, scale=float(D) ** -0.5)
                if diag:
                    # keep exp(score) where local query col >= key row
                    nc.gpsimd.affine_select(
                        out=pt[:, :, 0:w], in_=pt[:, :, 0:w],
                        compare_op=mybir.AluOpType.is_ge, fill=0.0,
                        base=0, channel_multiplier=-1, pattern=[[0, 2], [1, w]])
                for h in range(HPC):
                    nc.tensor.matmul(
                        pvt[:, h, off:QB],
                        lhsT=v_sb[:, h, b_, kt, :],
                        rhs=pt[:, h, 0:w],
                        start=(kt == 0), stop=(kt == n_kt - 1))

            def emit_norm(tb, pvt, tail=False):
                b_, qb = tb // NQB, tb % NQB
                cols = slice(b_ * T + qb * QB, b_ * T + (qb + 1) * QB)
                # Release the PSUM pair fast: pull denominators and the
                # UNNORMALIZED outputs (scaled 1/64 to stay in fp16 range)
                # to SBUF right away, then normalize in place off the
                # critical path (rb holds 64/den).
                d64 = stg.tile([D + 1, 2 * QB], F32, tag="d64", name="d64")
                for h in range(HPC):
                    nc.vector.tensor_scalar_mul(
                        d64[D:D + 1, h * QB:(h + 1) * QB],
                        pvt[D:D + 1, h, :], 1.0 / 64.0)
                nc.vector.tensor_scalar_mul(
                    attnTc[0:D, cols], pvt[0:D, 0, :], 1.0 / 64.0)
                nc.scalar.mul(out=attnT1[:, cols], in_=pvt[0:D, 1, :],
                              mul=1.0 / 64.0)
                nc.gpsimd.dma_start(
                    out=attnTc[D:2 * D, cols], in_=attnT1[:, cols])
                den0 = stg.tile([1, 2 * QB], F32, tag="den0", name="den0")
                nc.gpsimd.dma_start(out=den0, in_=d64[D:D + 1, :])
                rec0 = stg.tile([1, 2 * QB], F32, tag="rec0", name="rec0")
                nc.vector.reciprocal_approx_fast(out=rec0, in_=den0)
                r16 = stg.tile([1, 2 * QB], F16, tag="r16", name="r16")
                nc.vector.tensor_copy(out=r16, in_=rec0)
                if tail:
                    # broadcast 64/den via PE (ones-column outer product):
                    # faster than the gpsimd broadcast and keeps PE warm
                    rbps = psS.tile([128, 2 * QB], F32, tag="ps", name="rbps")
                    for h in range(HPC):
                        nc.tensor.matmul(
                            rbps[:, h * QB:(h + 1) * QB], lhsT=ones_row,
                            rhs=r16[:, h * QB:(h + 1) * QB],
                            start=True, stop=True)
                    rb0, rb1 = rbps[0:D, 0:QB], rbps[D:2 * D, QB:2 * QB]
                else:
                    rb = rbp.tile([128, 2 * QB], F16, tag="rb", name="rb")
                    nc.gpsimd.partition_broadcast(rb, r16)
                    rb0, rb1 = rb[0:D, 0:QB], rb[D:2 * D, QB:2 * QB]
                nc.vector.tensor_mul(
                    attnTc[0:D, cols], attnTc[0:D, cols], rb0)
                nc.vector.tensor_mul(
                    attnTc[D:2 * D, cols], attnTc[D:2 * D, cols], rb1)

            def emit_oproj(tb, i, tail=False, on_act=False):
                tt = tb * (QB // 128) + i // 2
                ch = i % 2
                if tail:
                    po = psS.tile([128, QB], F32, tag="ps", name="po")
                else:
                    po = psA.tile([128, QB], F32, tag="pa", name="po")
                nc.tensor.matmul(
                    po,
                    lhsT=attnTc[:, tt * 128:(tt + 1) * 128],
                    rhs=wout_sb[:, ch * QB:(ch + 1) * QB],
                    start=True, stop=True)
                ob = obp.tile([128, QB], F16, tag="ob", name="ob")
                if on_act or (tail and i % 2 == 1):
                    nc.scalar.copy(out=ob, in_=po)
                else:
                    nc.vector.tensor_copy(out=ob, in_=po)
                nc.sync.dma_start(
                    out=outp[tt * 128:(tt + 1) * 128, ch * QB:(ch + 1) * QB],
                    in_=ob)

            # ---- software-pipelined emission
            # prologue: projections for block 0
            for m in range(2):
                emit_qk(0, m)
            for c4 in range(QB // 128):
                emit_vT(0, c4)
            pend_po = []
            for tb in range(NTB):
                b_, qb = tb // NQB, tb % NQB
                if tb + 2 < NTB:
                    emit_xt_dma(tb + 2)
                n_kt = (qb + 1) * (QB // KT)
                short = n_kt <= 8
                pvt = psPV.tile([D + 1, 2, QB], F32, tag="pv", name="pv")
                # fill work interleaved between rounds.  Projections of the
                # next block go first (their inputs are long-ready, and the
                # next section's first rounds need them); out-proj of older
                # blocks goes late (it waits on the freshly computed
                # normalization).  In short sections excess out-proj work is
                # deferred and PSUM->SBUF drains move to the idle ACT engine.
                fills = []
                if tb + 1 < NTB:
                    fills += [(emit_qk, (tb + 1, m, short)) for m in range(2)]
                    fills += [(emit_vT, (tb + 1, c4))
                              for c4 in range(QB // 128)]
                if tb > 0:
                    pend_po += [(tb - 1, i) for i in range(8)]
                    cap = 6 if tb == NTB - 1 else max(0, n_kt - 3)
                    take, pend_po = pend_po[:cap], pend_po[cap:]
                    fills += [(emit_oproj, (ptb, i, False, short))
                              for ptb, i in take]
                nf = len(fills)
                # diag rounds (last 4) carry double fill weight: their PE
                # work is small but their exp+mask latency is full-size
                wts = [1] * (n_kt - 4) + [2] * 4
                tot = sum(wts)
                cum = 0
                for j in range(n_kt):
                    emit_round(b_, qb, j, pvt, n_kt)
                    lo = nf * cum // tot
                    cum += wts[j]
                    hi = nf * cum // tot
                    for fn, args in fills[lo:hi]:
                        fn(*args)
                emit_norm(tb, pvt, tail=(tb == NTB - 1))
            pend_po += [(NTB - 1, i) for i in range(8)]
            for ptb, i in pend_po:
                emit_oproj(ptb, i, tail=True)
    nc.compile()
    return nc


def _prep_in_maps(x, w_qkv, b_qkv, w_out):
    xTfull = np.ascontiguousarray(x.reshape(BT, C).T.astype(np.float16))
    in_maps = []
    for core in range(NCORES):
        hs = [core * HPC + i for i in range(HPC)]
        wq = np.ascontiguousarray(np.concatenate(
            [w_qkv[:, sec * C + h * D: sec * C + (h + 1) * D]
             for sec in range(3) for h in hs], axis=1).astype(np.float16))
        bqk_ = np.ascontiguousarray(np.stack(
            [np.concatenate([b_qkv[sec * C + h * D: sec * C + (h + 1) * D]
                             for h in hs])
             for sec in range(2)], axis=1).astype(np.float32))
        bv_ = np.ascontiguousarray(np.concatenate(
            [b_qkv[2 * C + h * D: 2 * C + (h + 1) * D] for h in hs]
        ).astype(np.float16).reshape(1, 128))
        wo = np.ascontiguousarray(np.concatenate(
            [w_out[h * D:(h + 1) * D, :] for h in hs], axis=0).astype(np.float16))
        in_maps.append({"xT": xTfull, "wqkv": wq, "bqk": bqk_, "bv16": bv_,
                        "wout": wo})
    return in_maps


LAST_RESULTS = None


def kernel(x, w_qkv, b_qkv, w_out, b_out):
    global LAST_RESULTS
    x = np.asarray(x, np.float32)
    w_qkv = np.asarray(w_qkv, np.float32)
    b_qkv = np.asarray(b_qkv, np.float32)
    w_out = np.asarray(w_out, np.float32)
    b_out = np.asarray(b_out, np.float32)

    if "nc" not in _cache:
        _cache["nc"] = _build_bass()
    nc = _cache["nc"]

    in_maps = _prep_in_maps(x, w_qkv, b_qkv, w_out)
    res = bass_utils.run_bass_kernel_spmd(nc, in_maps, core_ids=list(range(NCORES)))
    LAST_RESULTS = res

    out = res.results[0]["outp"].astype(np.float32)
    for r_ in res.results[1:]:
        out += r_["outp"].astype(np.float32)
    out += b_out
    return out.reshape(B, T, C)
